# revision 19
# baseline (speedup 1.0000x reference)
"""Trainium2 Bass kernel for pairwise radial-angular graph convolution.

Computes, for z in 0..3 (batch), a,b in 0..511 (points), i,j in 0..15:
  rel = g[z,b] - g[z,a];  d = sqrt(|rel|^2 + eps)
  rad_r = exp(-gamma*(d - c_r)^2)          (8 radial shells)
  ang   = [1, rel/d]                        (4 angular fns)
  out[z,a,i] = 1/sqrt(n) * sum_{b,r,m,j} rad_r*ang_m*W[r,m,i,j]*f[z,b,j]

v2 design (b-split, flipped bf16 contractions):
  8 cores = 4 z x 2 b-halves; each core computes partial sums over its 256
  neighbors for ALL 512 output points; host adds the two halves.

  Per core (2 b-tiles of 128, full a = 512):
    ndps[b,a] = -gamma*d^2            rank-6 f32r matmul into PSUM
    lt = ln(d^2+1e-5), d = exp(.5 lt) ACT (set-6 tables only: Ln/Exp/Square)
    rad'_r streams, bf16 [128, 8*512]:
      r=0:   e1 = exp(ndps)                     (ACT, from PSUM)
      r=1-5: m_r = m_{r-1} * u, u = exp(su*d)   (DVE TSP 4x chain; folded
             e^{+gamma c_r^2} keeps intermediates alive to d~3.75, beyond
             which these shells are < e-20)
      r=6:   exp(-gamma*Square(d - c6))         (ACT square + exp)
      r=7:   exp(2 gamma c7 d + ndps)           (DVE stt + ACT exp)
    rcpm = dmask / d                  (GPS divide; zero on the diagonal)
    q = rad * rcpm                    (DVE TSP 4x shells 0-6, GPS shell 7)
    contractions FLIPPED: stationary = rad/q [b,128-a-block] bf16,
      moving = G[b, 80-col shell block] bf16 -> psum[a,16|64] per a-block
      (cost = out cols: 8*(16+64)*4ab = 2560 cyc/tile vs 4096 unflipped)
    epilogue: out[a,i] = pS + pVb - sum_c g[a,c]*pV_c  (DVE TSPs with
      per-partition gA scalars)
"""

import math

import numpy as np

# ---------------------------------------------------------------- constants
Z, NPTS, C_IN, C_OUT = 4, 512, 16, 16
NUM_RADIAL, NUM_ANGULAR = 8, 4
MAX_R, GAMMA = 3.0, 8.0
N_CORES = 8
A = NPTS                        # full output width per core
B_PER_CORE = NPTS // 2          # 256 neighbors per core
N_BT = B_PER_CORE // 128        # 2 b-tiles of 128
CENTERS = [MAX_R * r / (NUM_RADIAL - 1) for r in range(NUM_RADIAL)]
SU = 2.0 * GAMMA * MAX_R / (NUM_RADIAL - 1)   # u = exp(su*d) chain step
LN_BIAS = 1e-4

# shell routes: 0 = e1, 1..5 = chain, 6 = ACT square, 7 = DVE stt
SQ_SHELLS = (6,)
STT_SHELLS = (7,)
CHAIN_SHELLS = (1, 2, 3, 4, 5)
# q-mul split: shells [0, N_QDVE) on DVE, rest on GPSIMD
N_QDVE = 5
CHAIN_TSP = False        # TensorTensor gets 2x DVE mode; 2-tensor TSP gets 1x

_CACHE = {}


def _build_program():
    import concourse.bacc as bacc
    import concourse.mybir as mybir
    import concourse.tile as tile

    f32 = mybir.dt.float32
    f32r = mybir.dt.float32r
    bf16 = mybir.dt.bfloat16
    i16 = mybir.dt.int16
    AF = mybir.ActivationFunctionType
    ALU = mybir.AluOpType

    nc = bacc.Bacc("TRN2", target_bir_lowering=False, debug=False)

    # register activation-bias constants (same pattern as Bass.__init__)
    bias_vals = {LN_BIAS} | {-CENTERS[r] for r in SQ_SHELLS}
    for v in sorted(bias_vals):
        t = nc.alloc_sbuf_tensor(f"const-f32-{v}", [128, 1], f32)
        nc.gpsimd.memset(t.ap(), v)
        nc.const_aps.aps[(f32, v)] = t.ap()
    nc.all_engine_barrier()

    # ---------------- IO -------------------------------------------------
    # fp32 (not f32r): the rank-6 distance matmul needs full fp32 precision
    # for the diagonal cancellation (f32r is ~2^-14 on the ~400-scale terms)
    b6_d = nc.dram_tensor("b6", [6, B_PER_CORE], f32, kind="ExternalInput")
    a6_d = nc.dram_tensor("a6", [6, A], f32, kind="ExternalInput")
    fext_d = nc.dram_tensor("fext", [64, B_PER_CORE], bf16,
                            kind="ExternalInput")
    wext_d = nc.dram_tensor("wext", [64, NUM_RADIAL * 80], bf16,
                            kind="ExternalInput")
    ga_d = nc.dram_tensor("ga", [128, 12], f32, kind="ExternalInput")
    dcol_d = nc.dram_tensor("dcol", [128, N_BT], f32, kind="ExternalInput")
    out_d = nc.dram_tensor("out", [128, 64], f32, kind="ExternalOutput")

    RW = 8 * A               # rad/q tile width (8 shells x 512)
    with tile.TileContext(nc) as tc:
        with (
            tc.tile_pool(name="const", bufs=1) as cpool,
            tc.tile_pool(name="gsb", bufs=N_BT) as gpool,
            tc.tile_pool(name="work", bufs=2) as wpool,
            tc.tile_pool(name="big", bufs=2) as bigpool,
            tc.tile_pool(name="fin", bufs=1) as fpool,
            tc.tile_pool(name="ndps", bufs=N_BT, space="PSUM") as ndpool,
            tc.tile_pool(name="gps", bufs=2, space="PSUM") as gppool,
            tc.tile_pool(name="accS", bufs=1, space="PSUM") as accS,
            tc.tile_pool(name="accV", bufs=1, space="PSUM") as accV,
        ):
            # ---------------- load inputs -------------------------------
            b6 = cpool.tile([6, B_PER_CORE], f32, tag="b6")
            a6 = cpool.tile([6, A], f32, tag="a6")
            fext = cpool.tile([64, B_PER_CORE], bf16, tag="fext")
            wext = cpool.tile([64, NUM_RADIAL * 80], bf16, tag="wext")
            ga = cpool.tile([128, 12], f32, tag="ga")
            dcol = cpool.tile([128, N_BT], f32, tag="dcol")
            for t, d in ((b6, b6_d), (a6, a6_d), (fext, fext_d),
                         (wext, wext_d), (ga, ga_d), (dcol, dcol_d)):
                nc.sync.dma_start(out=t[:], in_=d.ap())
            # iota16[p, c] = c - p: diagonal of b-tile t is where c-p == dcol_t
            iota16 = cpool.tile([128, A], i16, tag="iota16")
            nc.gpsimd.iota(iota16[:], pattern=[[1, A]], base=0,
                           channel_multiplier=-1)

            # pre-load the combined ln+exp+square activation table set once
            preload = mybir.InstLoadActFuncSet(
                name=nc.get_next_instruction_name(),
                act_func_set_id=6, ins=[], outs=[])
            preload.engine = mybir.EngineType.Activation
            nc.scalar.add_instruction(preload)

            # ---------------- ndps matmuls (both tiles up front) --------
            ndps = []
            for t in range(N_BT):
                bsl = slice(t * 128, (t + 1) * 128)
                nd = ndpool.tile([128, A], f32, tag="nd")
                nc.tensor.matmul(nd[:], b6[:, bsl], a6[:],
                                 start=True, stop=True)
                ndps.append(nd)

            # ---------------- G build: [128, r*80 + (S16|Vb16|V48)] -----
            g_sb = []
            for t in range(N_BT):
                bsl = slice(t * 128, (t + 1) * 128)
                gsb = gpool.tile([128, NUM_RADIAL * 80], bf16, tag="g")
                for h in range(2):
                    csl = slice(h * 320, (h + 1) * 320)
                    ps = gppool.tile([128, 320], f32, tag="gp")
                    nc.tensor.matmul(ps[:], fext[:, bsl], wext[:, csl],
                                     start=True, stop=True)
                    if h == 0:
                        nc.scalar.activation(gsb[:, csl], ps[:], AF.Copy)
                    else:
                        nc.vector.tensor_copy(gsb[:, csl], ps[:])
                g_sb.append(gsb)

            # ---------------- accumulators ------------------------------
            pS = accS.tile([128, 4 * 16], f32, tag="pS")
            pV = accV.tile([128, 4 * 64], f32, tag="pV")

            # ---------------- main loop over b-tiles --------------------
            for t in range(N_BT):
                nd = ndps[t]
                rad = bigpool.tile([128, RW], bf16, tag="rad")
                q = bigpool.tile([128, RW], bf16, tag="q")

                # ACT: e1 first (unblocks shell-0 matmuls), then d prologue
                nc.scalar.activation(rad[:, 0:A], nd[:], AF.Exp)
                lt = wpool.tile([128, A], f32, tag="lt")
                nc.scalar.activation(lt[:], nd[:], AF.Ln,
                                     bias=LN_BIAS, scale=-1.0 / GAMMA)
                dd = wpool.tile([128, A], f32, tag="dd")
                nc.scalar.activation(dd[:], lt[:], AF.Exp, scale=0.5)
                u = wpool.tile([128, A], bf16, tag="u")
                nc.scalar.activation(u[:], dd[:], AF.Exp, scale=SU)

                # DVE chain shells 1..5 (bf16 TSP 4x)
                for r in CHAIN_SHELLS:
                    psl = slice((r - 1) * A, r * A)
                    osl = slice(r * A, (r + 1) * A)
                    if CHAIN_TSP:
                        nc.vector.scalar_tensor_tensor(
                            rad[:, osl], rad[:, psl], 1.0, u[:],
                            ALU.mult, ALU.mult)
                    else:
                        nc.vector.tensor_mul(rad[:, osl], rad[:, psl], u[:])

                # shell 6: ACT square route (true rad, no fold)
                for r in SQ_SHELLS:
                    sq = wpool.tile([128, A], f32, tag="sq")
                    nc.scalar.activation(sq[:], dd[:], AF.Square,
                                         bias=-CENTERS[r])
                    nc.scalar.activation(rad[:, r * A:(r + 1) * A], sq[:],
                                         AF.Exp, scale=-GAMMA)
                # shell 7: DVE stt + ACT exp (folded)
                for r in STT_SHELLS:
                    arg = wpool.tile([128, A], f32, tag="arg")
                    nc.vector.scalar_tensor_tensor(
                        arg[:], dd[:], 2.0 * GAMMA * CENTERS[r], nd[:],
                        ALU.mult, ALU.add)
                    nc.scalar.activation(rad[:, r * A:(r + 1) * A], arg[:],
                                         AF.Exp)

                # rcpm = dmask / d  (diagonal-zero): DVE fast recip + GPS mask
                arcp = wpool.tile([128, A], f32, tag="arcp")
                nc.vector.reciprocal_approx_fast(out=arcp[:], in_=dd[:])
                neq = wpool.tile([128, A], bf16, tag="neq")
                nc.vector.tensor_scalar(neq[:], iota16[:],
                                        dcol[:, t:t + 1], None,
                                        ALU.not_equal)
                rcpm = wpool.tile([128, 1, A], bf16, tag="rcpm")
                nc.gpsimd.tensor_mul(rcpm[:, 0, :], arcp[:], neq[:])

                # q = rad * rcpm (broadcast): DVE shells [0, N_QDVE), GPS rest
                nq = N_QDVE
                if CHAIN_TSP:
                    nc.vector.scalar_tensor_tensor(
                        q[:, 0:nq * A].rearrange("p (r a) -> p r a", a=A),
                        rad[:, 0:nq * A].rearrange("p (r a) -> p r a", a=A),
                        1.0,
                        rcpm[:].to_broadcast([128, nq, A]),
                        ALU.mult, ALU.mult)
                else:
                    nc.vector.tensor_mul(
                        q[:, 0:nq * A].rearrange("p (r a) -> p r a", a=A),
                        rad[:, 0:nq * A].rearrange("p (r a) -> p r a", a=A),
                        rcpm[:].to_broadcast([128, nq, A]))
                if nq < NUM_RADIAL:
                    nc.gpsimd.tensor_mul(
                        q[:, nq * A:].rearrange("p (r a) -> p r a", a=A),
                        rad[:, nq * A:].rearrange("p (r a) -> p r a", a=A),
                        rcpm[:].to_broadcast([128, NUM_RADIAL - nq, A]))

                # ---- flipped contractions ------------------------------
                # start=True pending-zeroes the whole 2KB PSUM zero region,
                # so exactly ONE start (and one stop) per psum bank.
                first = t == 0
                last = t == N_BT - 1
                for r in range(NUM_RADIAL):
                    for ab in range(4):
                        st = rad[:, r * A + ab * 128: r * A + (ab + 1) * 128]
                        nc.tensor.matmul(
                            pS[:, ab * 16:(ab + 1) * 16], st,
                            g_sb[t][:, r * 80: r * 80 + 16],
                            start=(first and r == 0 and ab == 0),
                            stop=(last and r == NUM_RADIAL - 1 and ab == 3),
                            skip_group_check=True)
                for r in range(NUM_RADIAL):
                    for ab in range(4):
                        st = q[:, r * A + ab * 128: r * A + (ab + 1) * 128]
                        nc.tensor.matmul(
                            pV[:, ab * 64:(ab + 1) * 64], st,
                            g_sb[t][:, r * 80 + 16:(r + 1) * 80],
                            start=(first and r == 0 and ab == 0),
                            stop=(last and r == NUM_RADIAL - 1 and ab == 3),
                            skip_group_check=True)

            # ---------------- epilogue ----------------------------------
            # out[a,i] = pS + pVb + sum_c (-g[a,c]) * pV_c
            t1s = fpool.tile([128, 64], f32, tag="t1")
            t2s = fpool.tile([128, 64], f32, tag="t2")
            osb = fpool.tile([128, 64], f32, tag="osb")
            cS = fpool.tile([128, 64], f32, tag="cS")
            cV = fpool.tile([128, 256], f32, tag="cV")
            nc.scalar.activation(cS[:], pS[:], AF.Copy)
            nc.scalar.activation(cV[:], pV[:], AF.Copy)
            for ab in range(4):
                o = slice(ab * 16, (ab + 1) * 16)
                vb = slice(ab * 64, ab * 64 + 16)
                nc.vector.tensor_add(t1s[:, o], cS[:, o], cV[:, vb])
                nc.vector.scalar_tensor_tensor(
                    t2s[:, o], cV[:, ab * 64 + 16: ab * 64 + 32],
                    ga[:, 3 * ab: 3 * ab + 1], t1s[:, o],
                    ALU.mult, ALU.add)
                nc.vector.scalar_tensor_tensor(
                    t1s[:, o], cV[:, ab * 64 + 32: ab * 64 + 48],
                    ga[:, 3 * ab + 1: 3 * ab + 2], t2s[:, o],
                    ALU.mult, ALU.add)
                nc.vector.scalar_tensor_tensor(
                    osb[:, o], cV[:, ab * 64 + 48: ab * 64 + 64],
                    ga[:, 3 * ab + 2: 3 * ab + 3], t1s[:, o],
                    ALU.mult, ALU.add)
            nc.sync.dma_start(out=out_d.ap(), in_=osb[:])

    nc.compile()
    return nc


def _host_prep(features, geometry, W, n_norm):
    """Build per-core input maps (all small host-side tensors)."""
    import ml_dtypes
    bf = ml_dtypes.bfloat16

    f = np.asarray(features, dtype=np.float32)
    g = np.asarray(geometry, dtype=np.float32)
    W = np.asarray(W, dtype=np.float32)
    scale = 1.0 / math.sqrt(float(n_norm))

    # fold exp(-gamma c_r^2) (chain/e1/stt shells) and 1/sqrt(n) into W
    Wp = W.astype(np.float64) * scale
    for r in range(NUM_RADIAL):
        if r not in SQ_SHELLS:
            Wp[r] *= math.exp(-GAMMA * CENTERS[r] ** 2)
    Wp = Wp.astype(np.float32)

    # wext [64, r*80 + (S16 | Vb16 | V48)]:
    #   S  cols: rows 0:16 (j)        = Wp[r, 0, i, j]
    #   Vb cols: rows 16+16c+j        = Wp[r, c+1, i, j]
    #   V  cols: rows 0:16 (j)        = Wp[r, c+1, i, j] at col 16c+i
    wext = np.zeros((64, NUM_RADIAL * 80), dtype=np.float32)
    for r in range(NUM_RADIAL):
        base = r * 80
        wext[0:16, base:base + 16] = Wp[r, 0].T                  # [j, i]
        for c in range(3):
            wext[16 + 16 * c:32 + 16 * c, base + 16:base + 32] = Wp[r, c + 1].T
            wext[0:16, base + 32 + 16 * c:base + 48 + 16 * c] = Wp[r, c + 1].T

    in_maps = []
    for core in range(N_CORES):
        z, half = core // 2, core % 2
        b0 = half * B_PER_CORE
        gz = g[z]                                    # [512, 3]
        fz = f[z]                                    # [512, 16]
        gb = gz[b0:b0 + B_PER_CORE]                  # [256, 3]
        fb = fz[b0:b0 + B_PER_CORE]                  # [256, 16]

        b6 = np.empty((6, B_PER_CORE), dtype=np.float32)
        b6[0:3] = gb.T
        b6[3] = (gb * gb).sum(axis=1)
        b6[4] = 1.0
        b6[5] = 0.0

        a6 = np.empty((6, A), dtype=np.float32)
        a6[0:3] = 2.0 * GAMMA * gz.T
        a6[3] = -GAMMA
        a6[4] = -GAMMA * (gz * gz).sum(axis=1)
        a6[5] = 1.0

        fext = np.empty((64, B_PER_CORE), dtype=np.float32)
        fext[0:16] = fb.T
        for c in range(3):
            fext[16 + 16 * c:32 + 16 * c] = (fb * gb[:, c:c + 1]).T

        dcol = np.empty((128, N_BT), dtype=np.float32)
        for t in range(N_BT):
            dcol[:, t] = b0 + t * 128

        ga = np.empty((128, 12), dtype=np.float32)
        for ab in range(4):
            ga[:, 3 * ab:3 * ab + 3] = -gz[ab * 128:(ab + 1) * 128]

        in_maps.append({
            "b6": b6, "a6": a6,
            "fext": np.ascontiguousarray(fext).astype(bf),
            "wext": wext.astype(bf), "dcol": dcol, "ga": ga,
        })
    return in_maps


def kernel(features, geometry, W, n_norm):
    from concourse.bass_utils import run_bass_kernel_spmd

    if "nc" not in _CACHE:
        _CACHE["nc"] = _build_program()
    nc = _CACHE["nc"]

    in_maps = _host_prep(features, geometry, W, n_norm)
    res = run_bass_kernel_spmd(nc, in_maps, list(range(N_CORES)))

    out = np.zeros((Z, NPTS, C_OUT), dtype=np.float32)
    for core in range(N_CORES):
        z = core // 2
        o = np.asarray(res.results[core]["out"], dtype=np.float32)  # [128, 64]
        out[z] += o.reshape(128, 4, 16).transpose(1, 0, 2).reshape(NPTS, C_OUT)
    return out
